# revision 30
# baseline (speedup 1.0000x reference)
# Trainium2 Bass kernel for nn_DiT_89086211653924 (windowed-attention video DiT).
#
# Sharding (8 cores, zero collectives): core c -> (batch b = c//4, temporal
# group t = c%4) = frames [4t, 4t+4), i.e. 1024 of the 4096 tokens of batch b.
# Both the spatial windows (one frame = 256 contiguous tokens) and the
# spatio-temporal 4x4x4 windows (within one temporal group) are core-local, and
# the action-context cross-attention K/V is a per-batch constant, so each core
# runs the full two-block transformer on its 1024 tokens independently.
#
# On-device layout: activations are feature-major [c_in partition (128), c_tile
# (4), token (1024)] so every linear is matmul(lhsT=W[k_tile, c_out_slice],
# rhs=xT[k_tile, tokens]) with no transposes anywhere.  Value projections emit
# token-major V by using the activation tile as the stationary operand instead.
# LayerNorm gamma/beta are folded into the following matmul's weights+bias on
# the host.  On-device LN only computes z = h * rsqrt(var+eps) (per 512-token
# half, so consumers start early); the -mu*r part is applied inside each
# consumer as a rank-1 K=1 matmul psi_t * colsum(W)_c accumulated into the
# same PSUM group (see wbars / psirow).
#
# Dtypes: the residual stream h and all fp32 tensors feeding matmuls are
# typed float32r (TF32-like PE mode: same 4-byte layout, 4x throughput for
# free dims >= 256; bit-exact fp32 on DVE/ACT).  LN outputs z, q/k/v,
# exp(scores) and the context K/V are bf16 (full-rate PE at any free dim,
# half the SBUF).  PSUM stays fp32.  Walrus requires matmul operand dtypes
# to match when either is fp32/f32r, and requires producers of f32r-matmul
# inputs to be typed f32r themselves - hence the dtype plumbing throughout.
import sys

sys.path.insert(0, "/opt/trn_rl_repo")

import numpy as np

import concourse.bacc as bacc
import concourse.mybir as mybir
import concourse.tile as tile
from concourse.bass_utils import run_bass_kernel_spmd

F32 = mybir.dt.float32
AF = mybir.ActivationFunctionType
ALU = mybir.AluOpType
F32R = mybir.dt.float32r
BF16 = mybir.dt.bfloat16


def _mm(nc, out, lhsT, rhs, **kw):
    """Matmul with fp32 operands reinterpreted as float32r (TF32-like):
    same 4-byte layout, 4x the PE throughput for free dims >= 256.
    Non-fp32 operands (e.g. bf16 rows) pass through unchanged."""
    if lhsT.dtype == mybir.dt.float32:
        lhsT = lhsT.bitcast(F32R)
    if rhs.dtype == mybir.dt.float32:
        rhs = rhs.bitcast(F32R)
    nc.tensor.matmul(out, lhsT, rhs, **kw)

B, CIN, F, IMG, P = 2, 3, 16, 64, 4
PD = IMG // P          # 16 patches per side
NP = PD * PD           # 256 patches per frame
C, NH = 512, 8
HD = C // NH           # 64
S = 16                 # action context length
SCALE = float(1.0 / np.sqrt(HD))
M = 1024               # tokens per core (4 frames)
CT = 4                 # c tiles of 128
EPS = 1e-5

_BUILD_CACHE = {}


# ---------------------------------------------------------------- host prep

def _fold_block(p, w):
    """Fold LN gamma/beta into the consuming projections for block prefix p.

    Returns dict of device arrays for this block."""
    g, b = w[p + "_ln_g"], w[p + "_ln_b"]          # [3, C]
    qkv1, wo1, bo1 = w[p + "_qkv1"], w[p + "_wo1"], w[p + "_bo1"]
    qkv2, wo2, bo2 = w[p + "_qkv2"], w[p + "_wo2"], w[p + "_bo2"]
    f1w, f1b = w[p + "_fc1w"], w[p + "_fc1b"]
    f2w, f2b = w[p + "_fc2w"], w[p + "_fc2b"]
    d = {}
    # self-attn: q (scaled by 1/sqrt(HD)), k, v from LN1(h)
    d["wq1"] = (g[0][:, None] * qkv1[0]) * SCALE
    d["bq1"] = (b[0] @ qkv1[0]) * SCALE
    d["wk1"] = g[0][:, None] * qkv1[1]
    d["bk1"] = b[0] @ qkv1[1]
    d["wv1"] = g[0][:, None] * qkv1[2]
    d["bv1"] = b[0] @ qkv1[2]
    d["wo1"], d["bo1"] = wo1, bo1
    # cross-attn: q from LN2(h) (scaled); k,v from raw ctx (no LN, no bias)
    d["wq2"] = (g[1][:, None] * qkv2[0]) * SCALE
    d["bq2"] = (b[1] @ qkv2[0]) * SCALE
    d["wk2"] = qkv2[1]
    d["wv2"] = qkv2[2]
    d["wo2"], d["bo2"] = wo2, bo2
    # mlp from LN3(h)
    d["wf1"] = g[2][:, None] * f1w
    d["bf1"] = b[2] @ f1w + f1b
    d["wf2"], d["bf2"] = f2w, f2b
    return d


def jnp_bf16(a):
    import ml_dtypes
    return np.asarray(a, dtype=ml_dtypes.bfloat16)


def _w4(a):
    """[512, N] k-major weight -> [4, 128, N] (k_tile, k_in_tile, N)."""
    return np.ascontiguousarray(a.reshape(4, 128, -1).astype(np.float32))


def _colscal(a):
    """[512] bias -> [128, 4] per-partition scalar layout (col = c_tile)."""
    return np.ascontiguousarray(a.reshape(4, 128).T.astype(np.float32))


def _mprime_index():
    """m' (block-g window-major order) -> m (frame-major order)."""
    a, bb, f, i, j = np.meshgrid(
        np.arange(4), np.arange(4), np.arange(4), np.arange(4), np.arange(4),
        indexing="ij",
    )
    return (f * 256 + (4 * a + i) * 16 + 4 * bb + j).reshape(-1)


def host_prep(inputs):
    """Full inputs -> (shared weight map, list of 8 per-core input maps)."""
    w = {k: np.asarray(v) for k, v in inputs.items()}
    x = w["x"].astype(np.float32)                  # (B, CIN, F, IMG, IMG)
    actions = np.asarray(w["actions"])             # (B, S) int

    shared = {}
    shared["wp"] = np.ascontiguousarray(
        w["conv_w"].reshape(C, CIN * P * P).T.astype(np.float32))   # [48, 512]
    shared["convb"] = _colscal(w["conv_b"])
    shared["sppos"] = np.ascontiguousarray(
        w["sp_pos"].T.reshape(4, 128, NP).astype(np.float32))       # [4,128,256]
    shared["wh"] = _w4(w["head_w"])                                 # [4,128,48]
    shared["bh"] = np.ascontiguousarray(
        w["head_b"].reshape(1, 48).astype(np.float32))
    for p in ("s", "g"):
        fb = _fold_block(p, w)
        for nm in ("wo1", "wk2", "wv2", "wo2", "wf2"):
            shared[p + nm] = _w4(fb[nm])
        for nm in ("wq1", "wk1", "wv1", "wq2", "wf1"):
            shared[p + nm] = jnp_bf16(_w4(fb[nm]))
        for nm in ("bq1", "bk1", "bo1", "bq2", "bo2", "bf1", "bf2"):
            shared[p + nm] = _colscal(fb[nm])
        # v biases enter via a K=1 matmul row (token-major output)
        shared[p + "bv1r"] = np.ascontiguousarray(
            fb["bv1"].reshape(1, C).astype(np.float32))

    # column sums of the LN-consuming projections: the -mu*r ("psi") part
    # of LN is applied as a rank-1 K=1 matmul psi_t * wbar_c accumulated
    # into each consumer's PSUM instead of materializing z = h*r + psi.
    # All rows at partition 0 ([1, 10, C]): matmul requires lhsT and rhs to
    # share a base partition, and psi lives at partition 0.
    fbs, fbg = _fold_block("s", w), _fold_block("g", w)
    wbars = np.stack([
        fbs["wq1"].sum(0), fbs["wk1"].sum(0), fbs["wv1"].sum(0),
        fbs["wq2"].sum(0), fbs["wf1"].sum(0),
        fbg["wq1"].sum(0), fbg["wk1"].sum(0), fbg["wv1"].sum(0),
        fbg["wq2"].sum(0), fbg["wf1"].sum(0)])
    shared["onesc"] = np.ones((128, 1), np.float32)
    shared["onesr"] = np.ones((1, 512), np.float32)
    shared["onesq2"] = np.ones((128, 128), np.float32)
    shared["onescb"] = np.ascontiguousarray(jnp_bf16(np.ones((128, 1))))
    shared["wbars"] = np.ascontiguousarray(
        jnp_bf16(wbars.reshape(1, 10, C)))

    per_core = []
    for c in range(8):
        b, t = c // 4, c % 4
        m = dict(shared)
        xs = x[b, :, 4 * t:4 * t + 4]              # (CIN, 4, IMG, IMG)
        xs = xs.reshape(CIN, 4, PD, P, PD, P)
        xs = xs.transpose(0, 3, 5, 1, 2, 4)        # (c, p, q, f, ph, pw)
        m["xpt"] = np.ascontiguousarray(
            xs.reshape(CIN * P * P, M).astype(np.float32))          # [48, 1024]
        m["tpos"] = np.ascontiguousarray(
            w["t_pos"][4 * t:4 * t + 4].T.reshape(4, 128, 4)
            .astype(np.float32))                                    # [4,128,4]
        ctx = (w["act_table"][actions[b]] + w["act_pos"][0]).astype(np.float32)
        m["ctxt"] = np.ascontiguousarray(ctx.T.reshape(4, 128, S))  # [4,128,16]
        per_core.append(m)
    return per_core


# ---------------------------------------------------------------- device build

class Dev:
    """Holds nc handles used across the builder functions."""


def _declare_inputs(nc):
    d = Dev()
    d.nc = nc
    mk = lambda name, shape: nc.dram_tensor(name, shape, F32,
                                            kind="ExternalInput")
    mkr = lambda name, shape: nc.dram_tensor(name, shape, F32R,
                                             kind="ExternalInput")
    d.xpt = mkr("xpt", [48, M])
    d.tpos = mk("tpos", [4, 128, 4])
    d.sppos = mk("sppos", [4, 128, NP])
    d.ctxt = mkr("ctxt", [4, 128, S])
    d.wp = mkr("wp", [48, C])
    d.convb = mk("convb", [128, 4])
    d.wh = mkr("wh", [4, 128, 48])
    d.wbars = nc.dram_tensor("wbars", [1, 10, C], mybir.dt.bfloat16,
                             kind="ExternalInput")
    d.onesc = mkr("onesc", [128, 1])
    d.onesr = mkr("onesr", [1, 512])
    d.onesq2 = mkr("onesq2", [128, 128])
    d.onescb = nc.dram_tensor("onescb", [128, 1], mybir.dt.bfloat16,
                              kind="ExternalInput")
    d.bh = mkr("bh", [1, 48])
    for p in ("s", "g"):
        for nm in ("wo1", "wk2", "wv2", "wo2", "wf2"):
            setattr(d, p + nm, mkr(p + nm, [4, 128, C]))
        for nm in ("wq1", "wk1", "wv1", "wq2", "wf1"):
            setattr(d, p + nm, nc.dram_tensor(p + nm, [4, 128, C],
                                              mybir.dt.bfloat16,
                                              kind="ExternalInput"))
        for nm in ("bq1", "bk1", "bo1", "bq2", "bo2", "bf1", "bf2"):
            setattr(d, p + nm, mk(p + nm, [128, 4]))
        setattr(d, p + "bv1r", mkr(p + "bv1r", [1, C]))
    d.out = nc.dram_tensor("out", [48, M], F32, kind="ExternalOutput")
    return d


def _load_w(d, pools, dram, name):
    """DMA a [4,128,N] weight into sbuf [128, 4, N]."""
    nc = d.nc
    n = dram.shape[2]
    dt = dram.dtype
    tag = "w" + str(n) + ("b" if dt == BF16 else "")
    t = pools.wpool.tile([128, 4, n], dt, name=name, tag=tag)
    nc.sync.dma_start(out=t, in_=dram.ap().transpose([1, 0, 2]))
    return t


def _load_small(d, pools, dram, name):
    nc = d.nc
    shape = list(dram.shape)
    if len(shape) == 3 and shape[0] == 1:
        t = pools.cpool.tile(shape, dram.dtype, name=name)
        nc.sync.dma_start(out=t, in_=dram.ap())
    elif len(shape) == 3:
        # [4, 128, N] dram -> [128, 4, N] sbuf (partition-major)
        t = pools.cpool.tile([shape[1], shape[0], shape[2]], dram.dtype,
                             name=name)
        nc.sync.dma_start(out=t, in_=dram.ap().transpose([1, 0, 2]))
    else:
        t = pools.cpool.tile(shape, dram.dtype, name=name)
        nc.sync.dma_start(out=t, in_=dram.ap())
    return t


def _linear_fm(d, pools, w_sb, x_sb, epilogue, wbar=None, psi=None):
    """Feature-major linear: for each (ms, ct) produce psum [128,512] =
    sum_kt w_sb[:,kt,ct*128:+128].T @ x_sb[:,kt,ms*512:+512], then call
    epilogue(ct, ms, psum).  ms-major so each 512-token half can start as
    soon as its LN output is ready.  If wbar/psi given, accumulates the
    rank-1 LN term psi_t * wbar_c as an extra K=1 matmul."""
    nc = d.nc
    for ms in range(2):
        msl = slice(ms * 512, (ms + 1) * 512)
        for ct in range(CT):
            ps = pools.ps.tile([128, 512], F32, name="lin_ps", tag="ps")
            for kt in range(CT):
                _mm(nc, 
                    ps,
                    w_sb[:, kt, ct * 128:(ct + 1) * 128],
                    x_sb[:, kt, msl],
                    start=(kt == 0), stop=(kt == 3 and wbar is None),
                )
            if wbar is not None:
                _mm(nc, ps, d.wbars_sb[:, wbar, ct * 128:(ct + 1) * 128],
                    psi[:, msl], start=False, stop=True)
            epilogue(ct, ms, ps)


def _proj_act(d, pools, w_sb, b_sb, x_sb, out_sb, wbar=None, psi=None):
    """Linear + per-c_out bias via ACT Identity, into feature-major out."""
    nc = d.nc

    def ep(ct, ms, ps):
        nc.scalar.activation(
            out=out_sb[:, ct, ms * 512:(ms + 1) * 512], in_=ps,
            func=AF.Identity, bias=b_sb[:, ct:ct + 1], scale=1.0)

    _linear_fm(d, pools, w_sb, x_sb, ep, wbar=wbar, psi=psi)


def _proj_residual(d, pools, w_sb, b_sb, x_sb, h_sb):
    """h += x @ W + b  (one fused DVE op per tile)."""
    nc = d.nc

    def ep(ct, ms, ps):
        sl = h_sb[:, ct, ms * 512:(ms + 1) * 512]
        nc.vector.scalar_tensor_tensor(
            out=sl, in0=ps, scalar=b_sb[:, ct:ct + 1], in1=sl,
            op0=ALU.add, op1=ALU.add)

    _linear_fm(d, pools, w_sb, x_sb, ep)


def _layernorm(d, pools, h_sb, z_sb, tag):
    """z = h * rsqrt(var_c(h) + eps), feature-major, emitted per 512-token
    half.  The -mu*r part of LN is NOT applied here: it is returned as the
    "psi" row [1, M] and accumulated by each consumer as a rank-1 K=1
    matmul psi_t * colsum(W)_c (see _linear_fm wbar/psi).

    Returns the rows tile; psi lives at rows[:, 3, :]."""
    nc = d.nc
    sq = pools.state.tile([128, CT, M], F32R, name="lnsq", tag="scratch16")
    rows = pools.rows.tile([1, 3, M], F32R, name="lnrows_" + tag,
                           tag="lnrows", bufs=1)
    psirow = pools.rows.tile([1, M], BF16, name="psirow_" + tag,
                             tag="psirow", bufs=2)
    rb = pools.state.tile([128, M], F32, name="ln_rb", tag="bc_r")
    MU, A, BR = range(3)     # A ends as r; MU ends as psi; BR is scratch
    for ms in range(2):
        msl = slice(ms * 512, (ms + 1) * 512)
        nc.scalar.activation(out=sq[:, :, msl], in_=h_sb[:, :, msl],
                             func=AF.Square)
        for which, src in ((0, h_sb), (1, sq)):
            ps = pools.ps.tile([128, 512], F32, name="ln_ps", tag="ps")
            for kt in range(CT):
                _mm(nc, ps[0:1, :], d.ones_col,
                                 src[:, kt, msl],
                                 start=(kt == 0), stop=(kt == 3))
            dst = MU if which == 0 else A
            nc.scalar.activation(out=rows[:, dst, msl], in_=ps[0:1, :],
                                 func=AF.Copy, scale=1.0 / C)
        nc.vector.tensor_mul(rows[:, BR, msl], rows[:, MU, msl],
                             rows[:, MU, msl])
        nc.vector.tensor_sub(rows[:, A, msl], rows[:, A, msl],
                             rows[:, BR, msl])
        nc.scalar.activation(out=rows[:, BR, msl], in_=rows[:, A, msl],
                             func=AF.Sqrt, bias=d.eps_row)
        nc.vector.reciprocal(rows[:, A, msl], rows[:, BR, msl])
        nc.vector.scalar_tensor_tensor(
            out=psirow[:, msl], in0=rows[:, MU, msl], scalar=-1.0,
            in1=rows[:, A, msl], op0=ALU.mult, op1=ALU.mult)
        # broadcast r across partitions via K=1 ones matmul; z = h * r for
        # this half so consumer matmuls can start while the other half runs
        ps = pools.ps.tile([128, 512], F32, name="bc_ps", tag="ps")
        _mm(nc, ps, d.ones_row[:, 0:128], rows[:, A, msl],
                         start=True, stop=True)
        nc.scalar.copy(out=rb[:, msl], in_=ps)
        rbv = rb[:, msl].unsqueeze(1).broadcast_to([128, CT, 512])
        nc.vector.tensor_mul(z_sb[:, :, msl], h_sb[:, :, msl], rbv)
    return psirow


def _v_proj_tokmajor(d, pools, w_sb, brow_sb, x_sb, v_sb, wbar=None,
                     psi=None):
    """Token-major value projection: v_sb[128 tok, mt, C] with bias and the
    rank-1 LN psi term applied via extra K=1 matmuls (token side is the
    stationary operand here, so psi is the lhsT)."""
    nc = d.nc
    for mt in range(8):
        mtl = slice(mt * 128, (mt + 1) * 128)
        ps = pools.ps.tile([128, 512], F32, name="v_ps", tag="ps")
        for kt in range(CT):
            _mm(nc, 
                ps, x_sb[:, kt, mtl],
                w_sb[:, kt, :], start=(kt == 0),
                stop=(kt == 3 and brow_sb is None and wbar is None))
        if brow_sb is not None:
            _mm(nc, ps, d.ones_row[:, 0:128], brow_sb,
                             start=False, stop=(wbar is None))
        if wbar is not None:
            _mm(nc, ps, psi[:, mtl], d.wbars_sb[:, wbar, :],
                start=False, stop=True)
        nc.scalar.copy(out=v_sb[:, mt, :], in_=ps)


def _self_attn_s(d, pools, z_sb, attn_sb, wbar=None, psi=None):
    """Block-s self attention: 4 windows (frames) x 256 tokens, 8 heads."""
    nc = d.nc
    v_sb = pools.state.tile([128, 8, C], BF16, name="v_s", tag="vbuf")
    _v_proj_tokmajor(d, pools, _load_w(d, pools, d.swv1, "wv1"),
                 d.sbv1r_sb, z_sb, v_sb, wbar=wbar[2], psi=psi)
    for w in range(4):
        for h in range(NH):
            hp, ct = 64 * (h % 2), h // 2
            krows = slice(hp, hp + 64)
            sc = pools.ps.tile([128, 2, 256], F32, name="sc", tag="ps")
            for st in range(2):
                _mm(nc, 
                    sc[:, st, :],
                    d.skt_sb[krows, ct, w * 256 + st * 128:
                             w * 256 + st * 128 + 128],
                    d.sqt_sb[krows, ct, w * 256:w * 256 + 256],
                    start=True, stop=True)
            ex = pools.epool.tile([128, 2, 256], BF16, name="ex", tag="ex")
            nc.scalar.activation(out=ex, in_=sc, func=AF.Exp)
            dn = pools.ps.tile([128, 512], F32, name="dn", tag="ps")
            ob = pools.ps.tile([128, 512], F32, name="ob", tag="ps")
            for st in range(2):
                _mm(nc, dn[0:1, 0:256], d.ones_col_bf, ex[:, st, :],
                                 start=(st == 0), stop=(st == 1))
                _mm(nc, 
                    ob[hp:hp + 64, 0:256],
                    v_sb[:, 2 * w + st, h * 64:h * 64 + 64],
                    ex[:, st, :], start=(st == 0), stop=(st == 1),
                    tile_position=(0, hp))
            # broadcast the denominator row, then one reciprocal moves it
            # PSUM->SBUF for all partitions (no separate copy)
            dnr = pools.rows.tile([1, 512], F32R, name="dnr", tag="rr")
            nc.scalar.copy(out=dnr[:, 0:256], in_=dn[0:1, 0:256])
            rb = pools.ps.tile([128, 512], F32, name="rb", tag="ps")
            _mm(nc, rb[:, 0:256], d.ones_row[:, 0:128], dnr[:, 0:256],
                             start=True, stop=True)
            rbs = pools.epool.tile([128, 256], F32, name="rbs", tag="rbs")
            nc.vector.reciprocal(rbs, rb[:, 0:256])
            nc.vector.tensor_mul(
                attn_sb[krows, ct, w * 256:w * 256 + 256],
                ob[hp:hp + 64, 0:256], rbs[hp:hp + 64, :])


def _self_attn_g(d, pools, z_sb, attn_sb, wbar=None, psi=None):
    """Block-g self attention: 16 windows x 64 tokens (m' order), 8 heads,
    processed as 8 window-pairs.

    PSUM hazard rule (HW-verified): within one bank, concurrent matmuls with
    different row groups but overlapping column strips fault.  So scores are
    banked by head parity (source rows fixed per bank) and o/denominator by
    window parity (rows = 64j fixed per bank)."""
    nc = d.nc
    v_sb = pools.state.tile([128, 8, C], BF16, name="v_g", tag="vbuf")
    _v_proj_tokmajor(d, pools, _load_w(d, pools, d.gwv1, "wv1"),
                 d.gbv1r_sb, z_sb, v_sb, wbar=wbar[2], psi=psi)
    if _STAGE == 33:
        nc.vector.memset(attn_sb, 0.0)
        return
    if 34 <= _STAGE <= 37:
        nc.vector.memset(attn_sb, 0.0)
    for wp in range(8):
        # scores: bank per head-parity p; window parity j selects out rows
        scs = [pools.ps.tile([128, 4, 64], F32, name=f"sc_g{p}", tag="ps")
               for p in range(2)]
        for h in range(NH):
            p, ct = h % 2, h // 2
            hp = 64 * p
            for j in range(2):
                w = 2 * wp + j
                _mm(nc, 
                    scs[p][64 * j:64 * j + 64, ct, :],
                    d.gkt_sb[hp:hp + 64, ct, w * 64:w * 64 + 64],
                    d.gqt_sb[hp:hp + 64, ct, w * 64:w * 64 + 64],
                    start=True, stop=True, tile_position=(hp, 64 * j))
        if _STAGE == 34:
            continue
        exs = []
        for p in range(2):
            ex = pools.epool.tile([128, 4, 64], BF16, name=f"ex_g{p}",
                                  tag="ex")
            nc.scalar.activation(out=ex, in_=scs[p], func=AF.Exp)
            exs.append(ex)
        if _STAGE == 35:
            continue
        # o and denominators: bank per window parity j (rows 64j fixed)
        ogs = [pools.ps.tile([128, 4, 64], F32, name=f"og{j}", tag="ps")
               for j in range(2)]
        dns = [pools.ps.tile([128, 512], F32, name=f"dn_g{j}", tag="ps")
               for j in range(2)]
        for h in range(NH):
            p, ct = h % 2, h // 2
            for j in range(2):
                jr = slice(64 * j, 64 * j + 64)
                _mm(nc, 
                    dns[j][0:1, h * 64:h * 64 + 64],
                    d.ones_col_bf[jr, :], exs[p][jr, ct, :],
                    start=True, stop=True, tile_position=(64 * j, 0))
                _mm(nc, 
                    ogs[j][64 * p:64 * p + 64, ct, :],
                    v_sb[jr, wp, h * 64:h * 64 + 64],
                    exs[p][jr, ct, :],
                    start=True, stop=True, tile_position=(64 * j, 64 * p))
        if _STAGE == 36:
            continue
        for j in range(2):
            rrow = pools.rows.tile([1, 512], F32R, name=f"rr_g{j}", tag="rr")
            nc.vector.reciprocal(rrow, dns[j][0:1, :])
            rb = pools.ps.tile([128, 512], F32, name=f"rb_g{j}", tag="ps")
            _mm(nc, rb, d.ones_row[:, 0:128], rrow,
                             start=True, stop=True)
            rbs = pools.epool.tile([128, 512], F32, name=f"rbs_g{j}",
                                   tag="rbs")
            nc.scalar.copy(out=rbs, in_=rb)
            if _STAGE == 37:
                continue
            for p in range(2):
                pr = slice(64 * p, 64 * p + 64)
                out_view = attn_sb[pr, :, (2 * wp + j) * 64:
                                   (2 * wp + j) * 64 + 64]
                in1 = rbs[pr, :].rearrange(
                    "q (ct two l) -> q ct two l", two=2, l=64)[:, :, p, :]
                nc.vector.tensor_mul(out_view, ogs[j][pr, :, :], in1)


def _cross_attn(d, pools, prefix, z_sb, attn_sb):
    """Cross attention to the 16 action-context tokens (shared by all
    windows).  Heads packed 4-per-PSUM-bank at 32-partition strips."""
    nc = d.nc
    wk = _load_w(d, pools, getattr(d, prefix + "wk2"), "wk2")
    wv = _load_w(d, pools, getattr(d, prefix + "wv2"), "wv2")
    qt = getattr(d, prefix + "qt2_sb")
    # K/V from ctx (tiny): kT [128, 4, 16] feature-major; V [16, C] tok-major
    ktc = pools.state.tile([128, 4, S], BF16, name=prefix + "ktc", tag="ktc")
    for ct in range(CT):
        ps = pools.ps.tile([128, 512], F32, name="ktc_ps", tag="ps")
        for kt in range(CT):
            _mm(nc, ps[:, 0:S], wk[:, kt, ct * 128:(ct + 1) * 128],
                             d.ctxt_sb[:, kt, :], start=(kt == 0),
                             stop=(kt == 3))
        nc.scalar.copy(out=ktc[:, ct, :], in_=ps[:, 0:S])
    vc = pools.state.tile([128, 8, 64], BF16, name=prefix + "vc", tag="vc")
    ps = pools.ps.tile([128, 512], F32, name="vc_ps", tag="ps")
    for kt in range(CT):
        _mm(nc, ps[0:S, :], d.ctxt_sb[:, kt, :], wv[:, kt, :],
                         start=(kt == 0), stop=(kt == 3))
    vcv = vc.rearrange("p h d -> p (h d)")
    nc.scalar.copy(out=vcv[0:S, :], in_=ps[0:S, :])
    for q in range(1, 4):
        nc.sync.dma_start(out=vcv[32 * q:32 * q + S, :], in_=vcv[0:S, :])
    for ms in range(2):
        msl = slice(ms * 512, (ms + 1) * 512)
        exs = []
        for grp in range(2):                       # heads 4*grp .. 4*grp+3
            sc = pools.ps.tile([128, 512], F32, name="sc_x", tag="ps")
            nc.vector.memset(sc, 0.0)
            for hh in range(4):
                h = 4 * grp + hh
                hp, ct = 64 * (h % 2), h // 2
                _mm(nc, 
                    sc[32 * hh:32 * hh + S, :],
                    ktc[64 * (h % 2):64 * (h % 2) + 64, ct, :],
                    qt[64 * (h % 2):64 * (h % 2) + 64, ct, msl],
                    start=True, stop=True,
                    tile_position=(hp, 32 * hh))
            ex = pools.epool.tile([128, 512], BF16, name="ex_x", tag="ex")
            nc.scalar.activation(out=ex, in_=sc, func=AF.Exp)
            exs.append(ex)
        for h in range(NH):
            grp, hh = h // 4, h % 4
            sr = slice(32 * hh, 32 * hh + S)
            hp, ct = 64 * (h % 2), h // 2
            dn = pools.ps.tile([128, 512], F32, name="dn_x", tag="ps")
            _mm(nc, dn[0:1, :], d.ones_col_bf[sr, :], exs[grp][sr, :],
                             start=True, stop=True,
                             tile_position=(32 * hh, 0))
            ob = pools.ps.tile([128, 512], F32, name="ob_x", tag="ps")
            _mm(nc, ob[hp:hp + 64, :], vc[sr, h, :],
                             exs[grp][sr, :], start=True, stop=True,
                             tile_position=(32 * hh, hp))
            dnr = pools.rows.tile([1, 512], F32R, name="dnr_x", tag="rr")
            nc.scalar.copy(out=dnr, in_=dn[0:1, :])
            rb = pools.ps.tile([128, 512], F32, name="rb_x", tag="ps")
            _mm(nc, rb, d.ones_row[:, 0:128], dnr, start=True, stop=True)
            rbs = pools.epool.tile([128, 512], F32, name="rbs_x", tag="rbs")
            nc.vector.reciprocal(rbs, rb)
            nc.vector.tensor_mul(attn_sb[64 * (h % 2):64 * (h % 2) + 64,
                                         ct, msl],
                                 ob[hp:hp + 64, :], rbs[hp:hp + 64, :])


def _block(d, pools, prefix, h_sb):
    nc = d.nc
    z = pools.state.tile([128, CT, M], BF16, name="z", tag="zbuf")
    attn = pools.state.tile([128, CT, M], F32R, name="attn", tag="attnbuf")
    qt = pools.state.tile([128, CT, M], BF16, name=prefix + "qt", tag="qbuf")
    kt_ = pools.state.tile([128, CT, M], BF16, name=prefix + "kt", tag="kbuf")
    setattr(d, prefix + "qt_sb", qt)
    setattr(d, prefix + "kt_sb", kt_)

    # ---- self attention
    wbar = {"s": [0, 1, 2, 3, 4], "g": [5, 6, 7, 8, 9]}[prefix]
    psi1 = _layernorm(d, pools, h_sb, z, prefix + "1")
    if prefix == "s" and _STAGE == 20:
        return _dbg_out(d, pools, z[0:48, 0, :])
    if prefix == "g" and _STAGE == 30:
        return _dbg_out(d, pools, z[0:48, 0, :])
    _proj_act(d, pools, _load_w(d, pools, getattr(d, prefix + "wq1"), "wq1"),
              getattr(d, prefix + "bq1_sb"), z, qt, wbar=wbar[0], psi=psi1)
    _proj_act(d, pools, _load_w(d, pools, getattr(d, prefix + "wk1"), "wk1"),
              getattr(d, prefix + "bk1_sb"), z, kt_, wbar=wbar[1],
              psi=psi1)
    if prefix == "g" and _STAGE == 31:
        return _dbg_out(d, pools, qt[0:48, 0, :])
    if prefix == "s" and _STAGE == 21:
        return _dbg_out(d, pools, qt[0:48, 0, :])
    if prefix == "s":
        _self_attn_s(d, pools, z, attn, wbar=wbar, psi=psi1)
    else:
        _self_attn_g(d, pools, z, attn, wbar=wbar, psi=psi1)
    if prefix == "g" and _STAGE == 32:
        return _dbg_out(d, pools, attn[0:48, 0, :])
    if prefix == "s" and _STAGE == 22:
        return _dbg_out(d, pools, attn[0:48, 0, :])
    _proj_residual(d, pools, _load_w(d, pools, getattr(d, prefix + "wo1"),
                                     "wo1"),
                   getattr(d, prefix + "bo1_sb"), attn, h_sb)

    # ---- cross attention
    if prefix == "s" and _STAGE == 23:
        return _dbg_out(d, pools, h_sb[0:48, 0, :])
    psi2 = _layernorm(d, pools, h_sb, z, prefix + "2")
    qt2 = pools.state.tile([128, CT, M], BF16, name=prefix + "qt2", tag="kbuf")
    setattr(d, prefix + "qt2_sb", qt2)
    _proj_act(d, pools, _load_w(d, pools, getattr(d, prefix + "wq2"), "wq2"),
              getattr(d, prefix + "bq2_sb"), z, qt2, wbar=wbar[3],
              psi=psi2)
    _cross_attn(d, pools, prefix, z, attn)
    _proj_residual(d, pools, _load_w(d, pools, getattr(d, prefix + "wo2"),
                                     "wo2"),
                   getattr(d, prefix + "bo2_sb"), attn, h_sb)
    if prefix == "s" and _STAGE == 24:
        return _dbg_out(d, pools, h_sb[0:48, 0, :])

    # ---- mlp
    psi3 = _layernorm(d, pools, h_sb, z, prefix + "3")
    gbuf = pools.state.tile([128, CT, M], F32R, name=prefix + "g",
                            tag="scratch16")

    def ep_gelu(ct, ms, ps):
        nc.scalar.activation(
            out=gbuf[:, ct, ms * 512:(ms + 1) * 512], in_=ps, func=AF.Gelu,
            bias=getattr(d, prefix + "bf1_sb")[:, ct:ct + 1], scale=1.0)

    _linear_fm(d, pools, _load_w(d, pools, getattr(d, prefix + "wf1"),
                                 "wf1"), z, ep_gelu, wbar=wbar[4],
               psi=psi3)
    _proj_residual(d, pools, _load_w(d, pools, getattr(d, prefix + "wf2"),
                                     "wf2"),
                   getattr(d, prefix + "bf2_sb"), gbuf, h_sb)


class Pools:
    pass


import os as _os
_STAGE = int(_os.environ.get("DIT_STAGE", "99"))


def _dbg_out(d, pools, t_sb):
    """Debug: write [48, 1024]-shaped slice of a state tile to out and stop."""
    nc = d.nc
    outT = pools.cpool.tile([48, M], F32, name="outT")
    nc.vector.tensor_copy(out=outT, in_=t_sb)
    nc.sync.dma_start(out=d.out.ap(), in_=outT)


def _body(d, pools):
    """One full forward pass for this core's 1024 tokens."""
    nc = d.nc

    # constants and small tensors first (the weight DMAs are emitted lazily
    # at their use sites so the shared DMA queue can never head-of-line block
    # a load that an earlier weight's consumer depends on)
    xpt = pools.cpool.tile([48, M], F32R, name="xpt_sb", tag="io48")
    nc.sync.dma_start(out=xpt, in_=d.xpt.ap())
    wp = pools.state.tile([48, C], F32R, name="wp_sb", tag="vc")
    nc.sync.dma_start(out=wp, in_=d.wp.ap())
    d.convb_sb = _load_small(d, pools, d.convb, "convb")
    nc.sync.dma_start(out=d.ones_col, in_=d.onesc.ap())
    nc.sync.dma_start(out=d.ones_col_bf, in_=d.onescb.ap())
    nc.sync.dma_start(out=d.ones_row, in_=d.onesr.ap())
    nc.sync.dma_start(out=d.onesq, in_=d.onesq2.ap())
    d.ctxt_sb = _load_small(d, pools, d.ctxt, "ctxt")
    tpos = _load_small(d, pools, d.tpos, "tpos")
    sppos = pools.state.tile([128, 4, NP], F32, name="sppos_sb",
                             tag="bc_r")
    nc.sync.dma_start(out=sppos, in_=d.sppos.ap().transpose([1, 0, 2]))
    for p in ("s", "g"):
        for nm in ("bq1", "bk1", "bo1", "bq2", "bo2", "bf1", "bf2"):
            setattr(d, p + nm + "_sb",
                    _load_small(d, pools, getattr(d, p + nm), p + nm))
        setattr(d, p + "bv1r_sb",
                _load_small(d, pools, getattr(d, p + "bv1r"), p + "bv1r"))
    d.bh_sb = _load_small(d, pools, d.bh, "bh")
    d.wbars_sb = pools.cpool.tile([1, 10, C], mybir.dt.bfloat16,
                                  name="wbars")
    nc.sync.dma_start(out=d.wbars_sb, in_=d.wbars.ap())

    h = pools.state.tile([128, CT, M], F32R, name="h", tag="hbuf")

    # ---- patch embedding + positional embeddings
    for ct in range(CT):
        for ms in range(2):
            msl = slice(ms * 512, (ms + 1) * 512)
            ps = pools.ps.tile([128, 512], F32, name="pe_ps", tag="ps")
            _mm(nc, ps, wp[:, ct * 128:(ct + 1) * 128],
                             xpt[:, msl], start=True, stop=True)
            # + conv_b (per-partition) + t_pos (per frame: 2 frames per ms)
            tp = tpos[:, ct, 2 * ms:2 * ms + 2]
            tpv = tp.unsqueeze(2).broadcast_to([128, 2, 256])
            nc.vector.scalar_tensor_tensor(
                out=h[:, ct, msl].rearrange("p (f l) -> p f l", f=2),
                in0=ps.rearrange("p (f l) -> p f l", f=2),
                scalar=d.convb_sb[:, ct:ct + 1], in1=tpv,
                op0=ALU.add, op1=ALU.add)
        # + sp_pos (same 256 values for each of the 4 frames)
        spv = sppos[:, ct, :].unsqueeze(1).broadcast_to([128, 4, NP])
        hv = h[:, ct, :].rearrange("p (f l) -> p f l", f=4)
        nc.vector.tensor_add(hv, hv, spv)

    if _STAGE <= 1:
        return _dbg_out(d, pools, h[0:48, 0, :])

    # ---- block s (spatial windows = frames)
    _block(d, pools, "s", h)
    if _STAGE <= 2:
        return _dbg_out(d, pools, h[0:48, 0, :])

    # ---- permute tokens to m' (window-major for the 4x4x4 windows)
    hg = pools.state.tile([128, CT, M], F32R, name="hg", tag="zbuf2")
    for ct in range(CT):
        for a in range(4):
            src = bass_view_perm(h, ct, a)
            nc.vector.tensor_copy(
                out=hg[:, ct, a * 256:(a + 1) * 256].rearrange(
                    "p (b f i j) -> p b f i j", b=4, f=4, i=4),
                in_=src)
    if _STAGE <= 3:
        return _dbg_out(d, pools, hg[0:48, 0, :])
    _block(d, pools, "g", hg)
    if _STAGE <= 4:
        return _dbg_out(d, pools, hg[0:48, 0, :])

    # ---- output head (m' order; host undoes the permutation)
    d.wh_sb = _load_w(d, pools, d.wh, "wh")
    outT = pools.cpool.tile([48, M], F32, name="outT", tag="io48")
    for ms in range(2):
        msl = slice(ms * 512, (ms + 1) * 512)
        ps = pools.ps.tile([128, 512], F32, name="hd_ps", tag="ps")
        for kt in range(CT):
            _mm(nc, ps[0:48, :], d.wh_sb[:, kt, :], hg[:, kt, msl],
                             start=(kt == 0), stop=False)
        _mm(nc, ps[0:48, :], d.bh_sb, d.ones_row,
                         start=False, stop=True)
        nc.scalar.copy(out=outT[:, msl], in_=ps[0:48, :])
    nc.sync.dma_start(out=d.out.ap(), in_=outT)


def bass_view_perm(h, ct, a):
    """View of h[:, ct, :] selecting m' block a*256..a*256+255 in (b,f,i,j)
    nested order: m = f*256 + (4a+i)*16 + 4b + j."""
    base = h[:, ct, :]
    return bass_ap_nest(base, a)


def bass_ap_nest(base, a):
    import concourse.bass as bass  # noqa
    v = base.rearrange("p (f ph pw) -> p f ph pw", f=4, ph=16)
    # dims: f(stride 256), ph=4a+i (stride 16), pw=4b+j (stride 1)
    v2 = v[:, :, 4 * a:4 * a + 4, :].rearrange(
        "p f i (b j) -> p b f i j", b=4)
    return v2


def _build_nc(loop_n=1):
    key = loop_n
    if key in _BUILD_CACHE:
        return _BUILD_CACHE[key]
    nc = bacc.Bacc("TRN2", target_bir_lowering=False, debug=False,
                   num_devices=8)
    d = _declare_inputs(nc)
    with tile.TileContext(nc) as tc:
        pools = Pools()
        import contextlib
        stack = contextlib.ExitStack()
        with stack:
            pools.ps = stack.enter_context(
                tc.tile_pool(name="ps", bufs=8, space="PSUM"))
            pools.wpool = stack.enter_context(
                tc.tile_pool(name="wpool", bufs=3))
            pools.cpool = stack.enter_context(
                tc.tile_pool(name="cpool", bufs=1))
            pools.state = stack.enter_context(
                tc.tile_pool(name="state", bufs=1))
            pools.epool = stack.enter_context(
                tc.tile_pool(name="epool", bufs=3))
            pools.rows = stack.enter_context(
                tc.tile_pool(name="rows", bufs=2))
            ones_col = pools.cpool.tile([128, 1], F32R, name="ones_col")
            ones_col_bf = pools.cpool.tile([128, 1], BF16,
                                           name="ones_col_bf")
            d.ones_col_bf = ones_col_bf
            ones_row = pools.cpool.tile([1, 512], F32R, name="ones_row")
            onesq = pools.cpool.tile([128, 128], F32R, name="onesq")
            d.onesq = onesq
            d.ones_col = ones_col
            d.ones_row = ones_row
            eps_row = pools.cpool.tile([1, 1], F32, name="eps_row")
            nc.vector.memset(eps_row, EPS)
            d.eps_row = eps_row
            # python-level unroll for timing variants (a tc.For_i back
            # edge wedges the device here; the unrolled NEFF is equivalent)
            # float32r tags fp32 data for the PE's fast rounded mode; DVE/ACT
            # ops on it are bit-exact fp32, so the low-precision guard is
            # over-conservative here (end-to-end checked vs the reference).
            with nc.allow_low_precision(reason="f32r is fp32 off the PE"):
                for _ in range(loop_n):
                    _body(d, pools)
    nc.compile()
    _BUILD_CACHE[key] = nc
    return nc


# ---------------------------------------------------------------- entry point

_MPRIME = None


def kernel(**inputs) -> np.ndarray:
    global _MPRIME
    if _MPRIME is None:
        _MPRIME = _mprime_index()
    nc = _build_nc(1)
    in_maps = host_prep(inputs)
    res = run_bass_kernel_spmd(nc, in_maps, core_ids=list(range(8)))
    # reassemble: per-core outT [48, 1024] in m' order -> full output
    out = np.zeros((B, CIN, F, IMG, IMG), np.float32)
    for c in range(8):
        b, t = c // 4, c % 4
        ot = res.results[c]["out"]                 # [48, M] m'-order
        om = np.empty_like(ot)
        om[:, _MPRIME] = ot                        # -> m order
        # om[(p,q,cc), (f,ph,pw)] -> out[b, cc, 4t+f, ph*4+p, pw*4+q]
        om = om.reshape(P, P, CIN, 4, PD, PD)
        om = om.transpose(2, 3, 4, 0, 5, 1)        # (cc, f, ph, p, pw, q)
        out[b, :, 4 * t:4 * t + 4] = om.reshape(CIN, 4, IMG, IMG)
    return out


if __name__ == "__main__":
    import reference
    ins = reference.setup_inputs()
    ins = {k: np.asarray(v) for k, v in ins.items()}
    exp = np.asarray(reference.reference(**ins))
    act = kernel(**ins)
    err = np.abs(act - exp).max() / (np.abs(exp).max() + 1e-12)
    print("Relative error:", err)

